# revision 6
# baseline (speedup 1.0000x reference)
"""MinusAttention kernel for Trainium2 (8 NeuronCores, Bass/Tile).

Math: score[i,j] = (w.q_i - w.k_j + b) / sqrt(E) with causal mask.
Within a softmax row i, the w.q_i and b terms are constant across j and
cancel, so

    weights[i,j] = g_j / sum_{j'<=i} g_j',   g_j = exp(-w.k_j / sqrt(E))
    out[i,:]     = (sum_{j<=i} g_j V[j,:]) / (sum_{j<=i} g_j)

i.e. a cumulative weighted sum of V -- O(S*E) per (b,h) instead of
O(L*S*E), and the output does not depend on queries at all.

Device kernel (per core, 4 of the 32 (b,h) pairs):
  - sk[s] = sum_e KT'[e,s] where KT' is host-prescaled by -w[e]/sqrt(E)
    (16 matmuls per pair: lhsT = KT' block [64,128], rhs = ones [64,1])
  - g = exp(sk) on ScalarE
  - W[s,0:64] = g[s]*V[s,:], W[s,64] = g[s] (tensor_scalar against a
    host-appended ones column)
  - block sums bs_k = ones_col.T @ W_k ([1,65] in PSUM; compute-engine
    partition bases must be quadrant-aligned, so we cannot read row 127
    of the cumulative result directly) -> SBUF, then exclusive prefix
    carry_k = sum_{k'<k} bs_k' via 15 small DVE adds
  - per 128-row block k: PSUM = Tri_ut @ W_k + ones_row @ carry_k, where
    Tri_ut[j,i] = 1 for j<=i gives the within-block prefix sum
  - out_k = PSUM[:,0:64] * reciprocal(PSUM[:,64])
"""

import os

import numpy as np

B, L, S, H, E = 4, 2048, 2048, 8, 64
NCORES = 8
PAIRS = (B * H) // NCORES  # (b,h) pairs per core
NBLK = S // 128  # 128-row blocks per pair
SCALE = np.float32(1.0 / np.sqrt(np.float32(E)))

# test.py can flip this to capture an NTFF profile; the harness never does.
TRACE = False
LAST_RESULTS = None

_compiled = None


def _build():
    from concourse import bacc
    import concourse.mybir as mybir
    import concourse.tile as tile
    from concourse.masks import make_upper_triangular

    f32 = mybir.dt.float32
    nc = bacc.Bacc("TRN2", target_bir_lowering=False, debug=False)

    # kt packs two pairs per 128-partition slab: partitions 0-63 = pair 2j,
    # 64-127 = pair 2j+1, already scaled by -w/sqrt(E) on the host.
    kt = nc.dram_tensor("kt", [2, 128, S], f32, kind="ExternalInput")
    # vg[p, partition, block, 0:64] = V, [..., 64] = 1.0
    vg = nc.dram_tensor("vg", [PAIRS, 128, NBLK, E + 1], f32, kind="ExternalInput")
    out = nc.dram_tensor("out", [PAIRS, 128, NBLK, E], f32, kind="ExternalOutput")

    with tile.TileContext(nc) as tc:
        with (
            tc.tile_pool(name="const", bufs=1) as cpool,
            tc.tile_pool(name="ktp", bufs=2) as ktp,
            tc.tile_pool(name="vgp", bufs=PAIRS) as vgp,
            tc.tile_pool(name="wgp", bufs=PAIRS) as wgp,
            tc.tile_pool(name="gp", bufs=PAIRS) as gp,
            tc.tile_pool(name="carryp", bufs=PAIRS) as carryp,
            tc.tile_pool(name="bsp", bufs=PAIRS) as bsp,
            tc.tile_pool(name="rp", bufs=8) as rp,
            tc.tile_pool(name="outp", bufs=PAIRS) as outp,
            tc.tile_pool(name="psk", bufs=2, space="PSUM") as pskp,
            tc.tile_pool(name="pbs", bufs=2, space="PSUM") as pbsp,
            tc.tile_pool(name="pout", bufs=4, space="PSUM") as poutp,
        ):
            tri = cpool.tile([128, 128], f32)
            make_upper_triangular(nc, tri[:], val=1.0, diag=True)
            ones_row = cpool.tile([1, 128], f32)
            nc.gpsimd.memset(ones_row[:], 1.0)
            ones_col = cpool.tile([128, 1], f32)
            nc.gpsimd.memset(ones_col[:], 1.0)

            kts = []
            for j in range(2):
                t = ktp.tile([128, S], f32, tag="kt")
                nc.sync.dma_start(out=t[:], in_=kt[j])
                kts.append(t)

            state = []
            for p in range(PAIRS):
                ktile = kts[p // 2]
                poff = 64 * (p % 2)
                vgt = vgp.tile([128, NBLK, E + 1], f32, tag="vg")
                nc.sync.dma_start(out=vgt[:], in_=vg[p])

                psk = pskp.tile([128, NBLK], f32, tag="psk")
                for k in range(NBLK):
                    nc.tensor.matmul(
                        psk[:, k : k + 1],
                        lhsT=ktile[poff : poff + 64, k * 128 : (k + 1) * 128],
                        rhs=ones_col[poff : poff + 64, :],
                        start=True,
                        stop=True,
                    )
                g = gp.tile([128, NBLK], f32, tag="g")
                nc.scalar.activation(g[:], psk[:], mybir.ActivationFunctionType.Exp)

                wg = wgp.tile([128, NBLK, E + 1], f32, tag="wg")
                for k in range(NBLK):
                    nc.vector.tensor_scalar_mul(wg[:, k, :], vgt[:, k, :], g[:, k : k + 1])

                # block column sums bs[k] = sum_j W_k[j,:], then the
                # exclusive prefix carry[k] = sum_{k'<k} bs[k']
                bs = bsp.tile([1, NBLK, E + 1], f32, tag="bs")
                for k in range(NBLK):
                    pb = pbsp.tile([1, E + 1], f32, tag="pb")
                    nc.tensor.matmul(pb[:], lhsT=ones_col[:], rhs=wg[:, k, :], start=True, stop=True)
                    nc.scalar.copy(bs[:, k, :], pb[:])
                carry = carryp.tile([1, NBLK, E + 1], f32, tag="carry")
                nc.gpsimd.memset(carry[:, 0, :], 0.0)
                for k in range(1, NBLK):
                    nc.vector.tensor_add(carry[:, k, :], carry[:, k - 1, :], bs[:, k - 1, :])

                ot = outp.tile([128, NBLK, E], f32, tag="out")
                state.append((wg, carry, ot))

            # Block-major over pairs keeps independent work between the
            # per-pair PSUM accumulation groups.
            for k in range(NBLK):
                for p in range(PAIRS):
                    wg, carry, ot = state[p]
                    ps = poutp.tile([128, E + 1], f32, tag="ps")
                    if k == 0:
                        nc.tensor.matmul(ps[:], lhsT=tri[:], rhs=wg[:, k, :], start=True, stop=True)
                    else:
                        nc.tensor.matmul(ps[:], lhsT=tri[:], rhs=wg[:, k, :], start=True, stop=False)
                        nc.tensor.matmul(ps[:], lhsT=ones_row[:], rhs=carry[:, k, :], start=False, stop=True)
                    r = rp.tile([128, 1], f32, tag="r")
                    nc.vector.reciprocal(r[:], ps[:, E : E + 1])
                    nc.scalar.mul(ot[:, k, :], ps[:, 0:E], mul=r[:])

            for p in range(PAIRS):
                nc.sync.dma_start(out=out[p], in_=state[p][2][:])

    nc.compile()
    return nc


def _get_compiled():
    global _compiled
    if _compiled is None:
        _compiled = _build()
    return _compiled


def prep_inputs(keys: np.ndarray, values: np.ndarray, w_score: np.ndarray):
    """Host-side reshard: returns in_maps (list of 8 dicts)."""
    keys = np.asarray(keys, dtype=np.float32)
    values = np.asarray(values, dtype=np.float32)
    w = np.asarray(w_score, dtype=np.float32)

    # [B,S,H,E] -> [B,H,E,S] -> [B*H, E, S], prescaled by -w/sqrt(E)
    ktw = keys.transpose(0, 2, 3, 1).reshape(B * H, E, S)
    ktw = ktw * (-SCALE * w)[None, :, None]

    # [B,S,H,E] -> [B,H,S,E] -> [B*H, NBLK, 128, E] -> + ones col -> p-major
    v = values.transpose(0, 2, 1, 3).reshape(B * H, NBLK, 128, E)
    vg = np.concatenate([v, np.ones((B * H, NBLK, 128, 1), np.float32)], axis=-1)
    vg = vg.transpose(0, 2, 1, 3)  # [B*H, 128, NBLK, E+1]

    in_maps = []
    for c in range(NCORES):
        in_maps.append(
            {
                "kt": np.ascontiguousarray(ktw[PAIRS * c : PAIRS * (c + 1)]).reshape(2, 128, S),
                "vg": np.ascontiguousarray(vg[PAIRS * c : PAIRS * (c + 1)]),
            }
        )
    return in_maps


def assemble_output(results) -> np.ndarray:
    # results[c]["out"]: [PAIRS, 128, NBLK, E]; s = 128*k + partition
    arr = np.stack([np.asarray(r["out"]) for r in results])  # [8, PAIRS, 128, NBLK, E]
    arr = arr.reshape(B * H, 128, NBLK, E).transpose(0, 2, 1, 3)  # [B*H, NBLK, 128, E]
    arr = arr.reshape(B, H, L, E).transpose(0, 2, 1, 3)  # [B, L, H, E]
    return np.ascontiguousarray(arr)


def kernel(queries=None, keys=None, values=None, w_score=None, b_score=None, attn_mask=None, **_):
    global LAST_RESULTS
    from concourse.bass_utils import run_bass_kernel_spmd

    nc = _get_compiled()
    in_maps = prep_inputs(keys, values, w_score)
    res = run_bass_kernel_spmd(nc, in_maps, core_ids=list(range(NCORES)), trace=TRACE)
    LAST_RESULTS = res
    return assemble_output(res.results)
